# revision 22
# baseline (speedup 1.0000x reference)
"""Multi-head attention (B=2, N=2048, D=1024, H=16) on 8 TRN2 NeuronCores.

Sharding: tensor-parallel over heads. Core c owns heads 2c, 2c+1 (a 128-wide
slice of the concat head dim). Each core:
  - projects Q^T, K^T (transposed layout [dh, rows]) and V (natural [rows, dh])
    for its heads, over all B*N=4096 rows, from host-transposed bf16 x^T inputs
  - attention with transposed scores S^T[k, q] = K Q^T (f32r matmuls), exp on
    ScalarE (scale=1/8 folded in, no max-subtract needed: |scores/8| < ~4),
    softmax denominator via an appended ones-column in V (free on TensorE),
  - partial output projection out^T_c = Wo[:, slice] X_c^T  ->  [1024, 4096]
Host sums the 8 partial outputs and adds bo.

Loop order: batch-0 projections, batch-0 attention, batch-1 projections,
batch-1 attention — so attention starts while the other batch projects.
PV psum accumulators are copied to SBUF immediately (frees the PSUM slot, PE
never idles long enough for the HAM clock-gate to re-throttle); the softmax
normalize chain (reciprocal/broadcast/multiply) runs off the critical path.
"""

import sys

sys.path.insert(0, "/opt/trn_rl_repo")

from contextlib import ExitStack

import ml_dtypes
import numpy as np

import concourse.bass as bass
import concourse.mybir as mybir
import concourse.tile as tile
from concourse import bacc
from concourse.bass_utils import run_bass_kernel_spmd

B, N, D, H, DH = 2, 2048, 1024, 16, 64
R = B * N  # 4096
NC = 8
HPC = H // NC  # 2 heads per core
DHC = HPC * DH  # 128 head dims per core
QT = 512  # query tile (psum bank / fp32 moving max)
KT = 128  # key tile (psum partitions)
NQT = N // QT  # 4
NKT = N // KT  # 16
NBRT = N // QT  # 4 row tiles per batch for projections
KC = D // 128  # 8 contraction chunks

f32 = mybir.dt.float32
f32r = mybir.dt.float32r
bf16 = mybir.dt.bfloat16

_cache = {}


def _fold(ap):
    # [D, X] dram -> [128, KC, X] partition-folded view for one-shot DMA
    return ap.rearrange("(a p) m -> p a m", p=128)


def _foldw(w):
    # [D, DHC] host weight -> [128, KC, DHC] partition-folded, contiguous
    return np.ascontiguousarray(w.reshape(KC, 128, DHC).transpose(1, 0, 2))


def build():
    if "nc" in _cache:
        return _cache["nc"]
    nc = bacc.Bacc("TRN2", target_bir_lowering=False, debug=False, num_devices=NC)
    xq = nc.dram_tensor("xqT", [D, R], bf16, kind="ExternalInput").ap()
    xk = nc.dram_tensor("xkT", [D, R], bf16, kind="ExternalInput").ap()
    xv = nc.dram_tensor("xvT", [D, R], bf16, kind="ExternalInput").ap()
    wq = nc.dram_tensor("wqT", [128, KC, DHC], bf16, kind="ExternalInput").ap()
    wk = nc.dram_tensor("wkT", [128, KC, DHC], bf16, kind="ExternalInput").ap()
    wv = nc.dram_tensor("wvT", [128, KC, DHC], bf16, kind="ExternalInput").ap()
    wo = nc.dram_tensor("woT", [DHC, D], bf16, kind="ExternalInput").ap()
    bq = nc.dram_tensor("bq", [DHC, 1], f32, kind="ExternalInput").ap()
    bk = nc.dram_tensor("bk", [DHC, 1], f32, kind="ExternalInput").ap()
    bv = nc.dram_tensor("bv", [1, DHC], bf16, kind="ExternalInput").ap()
    outT = nc.dram_tensor("outT", [D, R], bf16, kind="ExternalOutput").ap()

    with tile.TileContext(nc) as tc, ExitStack() as ctx:
        const = ctx.enter_context(tc.tile_pool(name="const", bufs=1))
        xpool = ctx.enter_context(tc.tile_pool(name="x", bufs=4))
        big = ctx.enter_context(tc.tile_pool(name="big", bufs=1))
        ppool = ctx.enter_context(tc.tile_pool(name="p", bufs=3))
        opool = ctx.enter_context(tc.tile_pool(name="o", bufs=4))
        npool = ctx.enter_context(tc.tile_pool(name="norm", bufs=3))
        ps_proj = ctx.enter_context(tc.tile_pool(name="psA", bufs=2, space="PSUM"))
        ps_s = ctx.enter_context(tc.tile_pool(name="psS", bufs=2, space="PSUM"))
        ps_pv = ctx.enter_context(tc.tile_pool(name="psPV", bufs=2, space="PSUM"))

        # ---- constants ----
        wq_sb = const.tile([128, KC, DHC], bf16, tag="wq")
        nc.sync.dma_start(wq_sb[:], wq)
        wk_sb = const.tile([128, KC, DHC], bf16, tag="wk")
        nc.sync.dma_start(wk_sb[:], wk)
        wv_sb = const.tile([128, KC, DHC], bf16, tag="wv")
        nc.sync.dma_start(wv_sb[:], wv)
        wo_sb = const.tile([128, D], bf16, tag="wo")
        nc.sync.dma_start(wo_sb[:], wo)
        bq_sb = const.tile([DHC, 1], f32, tag="bq")
        nc.sync.dma_start(bq_sb[:], bq)
        bk_sb = const.tile([DHC, 1], f32, tag="bk")
        nc.sync.dma_start(bk_sb[:], bk)
        bv_sb = const.tile([1, DHC], bf16, tag="bv")
        nc.sync.dma_start(bv_sb[:], bv)
        ones_r = const.tile([1, 128], bf16, tag="onesr")
        nc.vector.memset(ones_r[:], 1.0)

        # ---- per-batch persistent activations ----
        qTs, kTs, vs = [], [], []
        for b in range(B):
            qTs.append(big.tile([128, N], bf16, tag=f"qT{b}", name=f"qT{b}"))
            kTs.append(big.tile([128, N], bf16, tag=f"kT{b}", name=f"kT{b}"))
            v = big.tile([128, HPC * NKT, 128], bf16, tag=f"v{b}", name=f"v{b}")
            nc.vector.memset(v[:, :, 64:128], 1.0)
            vs.append(v)

        XW = 2 * QT  # 1024 rows per x tile (2KB dma descriptors)

        def proj_one(b, dst, xdram, w_sb, b_sb, split=False):
            # psum[dh2, r] = sum_d W^T[d, dh2] x^T[d, r]  (+ bias in the copy)
            for xi in range(N // XW):
                rlo = b * N + xi * XW
                xt = xpool.tile([128, KC, XW], bf16, tag="xqk", name="xqk")
                src_ap = _fold(xdram[:, rlo : rlo + XW])
                if split and xi == 0:
                    for kc in range(KC):
                        nc.sync.dma_start(xt[:, kc, :], src_ap[:, kc, :])
                else:
                    nc.sync.dma_start(xt[:], src_ap)
                for rl in range(XW // QT):
                    ps = ps_proj.tile([128, QT], f32, tag="proj", name="psqk")
                    for kc in range(KC):
                        nc.tensor.matmul(
                            ps[:],
                            w_sb[:, kc, :],
                            xt[:, kc, rl * QT : (rl + 1) * QT],
                            start=(kc == 0), stop=(kc == KC - 1),
                        )
                    rt = xi * (XW // QT) + rl
                    nc.vector.tensor_scalar_add(
                        dst[:, rt * QT : (rt + 1) * QT], ps[:], b_sb[:]
                    )

        def proj_v(b):
            # natural layout: psum[r, dh2] = sum_d x^T[d, r] W^T[d, dh2]
            for xi in range(N // XW):
                rlo = b * N + xi * XW
                xt = xpool.tile([128, KC, XW], bf16, tag="xqk", name="xv")
                nc.sync.dma_start(xt[:], _fold(xv[:, rlo : rlo + XW]))
                for rs in range(XW // 128):
                    ps = ps_proj.tile([128, DHC], f32, tag="proj", name="psv")
                    for kc in range(KC):
                        nc.tensor.matmul(
                            ps[:],
                            xt[:, kc, rs * 128 : (rs + 1) * 128],
                            wv_sb[:, kc, :],
                            start=(kc == 0), stop=False,
                        )
                    nc.tensor.matmul(
                        ps[:], ones_r[:], bv_sb[:], start=False, stop=True
                    )
                    kt = xi * (XW // 128) + rs  # key tile index within batch
                    for h in range(HPC):
                        nc.vector.tensor_copy(
                            vs[b][:, h * NKT + kt, 0:64],
                            ps[:, 64 * h : 64 * h + 64],
                        )

        def proj_k(b):
            proj_one(b, kTs[b], xk, wk_sb, bk_sb, split=(b == 0))

        def proj_q(b):
            proj_one(b, qTs[b], xq, wq_sb, bq_sb)

        def attention(b, xT, qts):
            for qt in qts:
                qs = slice(qt * QT, (qt + 1) * QT)
                pvs = [
                    ps_pv.tile([128, QT], f32, tag="pv", name=f"pv{h}")
                    for h in range(HPC)
                ]
                for kt in range(NKT):
                    ks = slice(kt * KT, (kt + 1) * KT)
                    sg = ps_s.tile([128, 2 * QT], f32, tag="sg", name="sg")
                    for h in range(HPC):
                        hp = slice(64 * h, 64 * h + 64)
                        nc.tensor.matmul(
                            sg[:, h * QT : (h + 1) * QT],
                            kTs[b][hp, ks],
                            qTs[b][hp, qs],
                            start=True, stop=True,
                        )
                    pt = ppool.tile([128, 2 * QT], bf16, tag="p", name="pt")
                    nc.scalar.activation(
                        pt[:], sg[:], mybir.ActivationFunctionType.Exp, scale=0.125
                    )
                    for h in range(HPC):
                        nc.tensor.matmul(
                            pvs[h][:],
                            vs[b][:, h * NKT + kt, :],
                            pt[:, h * QT : (h + 1) * QT],
                            start=(kt == 0), stop=(kt == NKT - 1),
                        )
                for h in range(HPC):
                    # copy to SBUF immediately -> frees the psum slot so the
                    # next q-tile's PV starts without a long PE stall
                    pvsb = npool.tile([65, QT], f32, tag="pvsb", name=f"pvsb{h}")
                    nc.vector.tensor_copy(pvsb[:], pvs[h][0:65, :])
                    # sumexp row sits at partition 64; shift to 0 via sbuf DMA
                    rc = npool.tile([1, QT], f32, tag="rc", name=f"rc{h}")
                    nc.sync.dma_start(rc[:], pvsb[64:65, :])
                    nc.vector.reciprocal_approx_fast(rc[:], rc[:])
                    rb = npool.tile([64, QT], f32, tag="rb", name=f"rb{h}")
                    nc.gpsimd.partition_broadcast(rb[:], rc[:])
                    if h == 0:
                        nc.vector.tensor_mul(xT[0:64, qs], pvsb[0:64, :], rb[:])
                    else:
                        tmp = npool.tile([64, QT], bf16, tag="tmp", name="tmp")
                        nc.vector.tensor_mul(tmp[:], pvsb[0:64, :], rb[:])
                        nc.sync.dma_start(xT[64:128, qs], tmp[:])
                last = b == B - 1 and qt == NQT - 1
                for ot in range(KC):
                    ps = ps_proj.tile([128, QT], f32, tag="proj", name="pso")
                    nc.tensor.matmul(
                        ps[:],
                        wo_sb[:, ot * 128 : (ot + 1) * 128],
                        xT[:, qs],
                        start=True, stop=True,
                    )
                    ob = opool.tile([128, QT], bf16, tag="o", name="ob")
                    if last and ot % 2 == 0:
                        nc.scalar.copy(ob[:], ps[:])
                    else:
                        nc.vector.tensor_copy(ob[:], ps[:])
                    nc.gpsimd.dma_start(
                        outT[
                            ot * 128 : (ot + 1) * 128,
                            b * N + qt * QT : b * N + (qt + 1) * QT,
                        ],
                        ob[:],
                    )

        xTs = [
            opool.tile([128, N], bf16, tag="xT", name=f"xT{b}", bufs=2)
            for b in range(B)
        ]
        proj_k(0)
        proj_v(0)
        proj_q(0)
        attention(0, xTs[0], range(0, 1))
        proj_k(1)
        attention(0, xTs[0], range(1, 2))
        proj_v(1)
        attention(0, xTs[0], range(2, 3))
        proj_q(1)
        attention(0, xTs[0], range(3, NQT))
        attention(1, xTs[1], range(0, NQT))

    nc.compile()
    _cache["nc"] = nc
    return nc


def kernel(x_q, x_k, x_v, Wq, bq, Wk, bk, Wv, bv, Wo, bo, _trace=False):
    x_q = np.asarray(x_q, dtype=np.float32)
    x_k = np.asarray(x_k, dtype=np.float32)
    x_v = np.asarray(x_v, dtype=np.float32)
    Wq, Wk, Wv, Wo = (np.asarray(w, dtype=np.float32) for w in (Wq, Wk, Wv, Wo))
    bq, bk, bv, bo = (np.asarray(v, dtype=np.float32) for v in (bq, bk, bv, bo))

    bf = ml_dtypes.bfloat16
    xqT = np.ascontiguousarray(x_q.reshape(R, D).T).astype(bf)
    xkT = np.ascontiguousarray(x_k.reshape(R, D).T).astype(bf)
    xvT = np.ascontiguousarray(x_v.reshape(R, D).T).astype(bf)

    in_maps = []
    for c in range(NC):
        s = slice(DHC * c, DHC * (c + 1))
        in_maps.append(
            {
                "xqT": xqT,
                "xkT": xkT,
                "xvT": xvT,
                "wqT": _foldw(Wq[s, :].T).astype(bf),
                "wkT": _foldw(Wk[s, :].T).astype(bf),
                "wvT": _foldw(Wv[s, :].T).astype(bf),
                "woT": np.ascontiguousarray(Wo[:, s].T).astype(bf),
                "bq": bq[s][:, None].copy(),
                "bk": bk[s][:, None].copy(),
                "bv": bv[s][None, :].astype(bf),
            }
        )

    nc = build()
    res = run_bass_kernel_spmd(nc, in_maps, core_ids=list(range(NC)), trace=_trace)
    total = np.zeros((D, R), dtype=np.float32)
    for c in range(NC):
        total += res.results[c]["outT"].astype(np.float32)
    out = total.T + bo[None, :]
    if _trace:
        kernel.last_exec_time_ns = res.exec_time_ns
    return out.reshape(B, N, D).astype(np.float32)
